# revision 12
# baseline (speedup 1.0000x reference)
"""Trainium2 Bass kernel for nn_DendriteOutput.

Math: out[b, o] = sum_{d<32} x[b, o*32+d] * weight[o, o*32+d] + bias[o]
(block-diagonal connectivity: only the diagonal 32-wide blocks of `weight`
are touched, so the kernel never reads the other 99.2% of the matrix).

Sharding (8 cores, tensor-parallel over out_dim):
  core k handles outputs [k*256, (k+1)*256) for the full batch, i.e. the
  x column-slab [:, k*8192:(k+1)*8192] (32 MB/core -> the dominant HBM
  traffic; measured stream rate ~400 GB/s/core -> ~87 us for the loads).

Engine/queue layout (per core), chosen so the x-load DMA FIFO never
carries an instruction that waits on compute (HWDGE DMAs execute FIFO
per issuing engine):
  sync HWDGE queue : ONLY the 8 x-tile loads [128, 8192] f32.
  gpsimd SWDGE     : wrep/brep preloads + the 8 y stores (idle queue;
                     its waits can't stall the load stream).
  ScalarE          : all f32->fp16 tile casts (pure compute, no DMAs;
                     measured ~1.5 us per [128, 8192] cast).
  DVE              : in-place fp16 multiply by the replicated diagonal-
                     weight strip, then a log-tree segmented reduction
                     32->16->8->4->2->1 (fp16 2x strided adds, last
                     level + bias add in f32).
First/last batch tiles are split into feature halves to shrink pipeline
fill and tail.
"""

import json

import numpy as np

import concourse.bass as bass
import concourse.bass_utils as _bass_utils
import concourse.mybir as mybir
from concourse.tile import TileContext
from concourse.bass_utils import run_bass_kernel_spmd

BATCH = 1024
OUT_DIM = 2048
DPC = 32
N_CORES = 8
O_PER = OUT_DIM // N_CORES          # 256 outputs per core
F_PER = O_PER * DPC                 # 8192 features per core
BT = 128                            # batch rows per tile (SBUF partitions)
N_BT = BATCH // BT                  # 8 batch tiles per core
ACT_TILES = set(range(N_BT))        # tiles whose cast runs on ScalarE (all:
                                    # measured ~1.5 us/tile, ACT is idle else)
SPLIT_TILES = {0, N_BT - 1}         # tiles split into feature halves

# ---------------------------------------------------------------------------
# Environment workarounds (in-process only; nothing on disk is modified).
#
# The walrus build in this container (a) needs --dge-levels to lower HWDGE
# DMAs with sem waits (otherwise they hit the V2 pseudo-DMA path that allows
# none) and (b) caps sync waits at ONE per instruction while Tile attaches up
# to N (e.g. the kernel-tail drain). We add the flag and rewrite the
# serialized BIR: extra waits are hoisted into preceding single-wait Drain
# carriers on the same engine (safe: a wait only moves earlier within the
# same engine-program order).
# ---------------------------------------------------------------------------

_patched = False


def _patch_walrus_flags():
    global _patched
    if _patched:
        return
    _patched = True
    orig_rc = _bass_utils.run_command

    def rc(cmd, cwd=None, **kw):
        if cmd and "walrus_driver" in str(cmd[0]):
            cmd = list(cmd)
            cmd.insert(1, "--dge-levels=io,spill_reload,scalar_dynamic_offset")
        return orig_rc(cmd, cwd=cwd, **kw)

    _bass_utils.run_command = rc


def _split_multi_waits(bir_bytes: bytes, cap: int = 1) -> bytes:
    m = json.loads(bir_bytes)
    for fn in m["functions"]:
        for blk in fn["blocks"]:
            out = []
            for inst in blk["instructions"]:
                si = inst.get("sync_info")
                waits = (si or {}).get("on_wait") or []
                if len(waits) > cap:
                    keep = waits[-cap:]
                    for j, wchunk in enumerate(waits[:-cap]):
                        out.append(
                            {
                                "debug": inst.get("debug"),
                                "engine": inst["engine"],
                                "ins": [],
                                "name": f"{inst['name']}-ws{j}",
                                "opcode": "Drain",
                                "outs": [],
                                "sync_info": {
                                    "on_update": [],
                                    "on_wait": [wchunk],
                                },
                            }
                        )
                    si["on_wait"] = keep
                out.append(inst)
            blk["instructions"] = out
    return json.dumps(m).encode()


def _emit_segment(nc, wpool, opool, wrep, brep, y, xt32, rep, i, half, c0, c1):
    """Multiply+reduce+store for columns [c0, c1) of batch tile i.

    xt32: the f32 x tile [128, F_PER] (full row block); we read its
    [:, c0:c1] slice. half: unique name suffix.
    """
    f32 = mybir.dt.float32
    f16 = mybir.dt.float16
    nf = c1 - c0
    no = nf // DPC
    ob = c0 // DPC
    q0 = wpool.tile([128, F_PER], f16, tag="q0", bufs=2,
                    name=f"q0_{rep}_{i}_{half}")
    q0s = q0[:, 0:nf]
    if i in ACT_TILES:
        nc.scalar.copy(q0s, xt32[:, c0:c1])
        nc.vector.tensor_mul(q0s, q0s, wrep[:, c0:c1])
    else:
        nc.vector.tensor_mul(q0s, xt32[:, c0:c1], wrep[:, c0:c1])
    p3 = q0s.rearrange("p (o d) -> p o d", d=DPC)
    q1 = wpool.tile([128, O_PER * 16], f16, tag="q1", bufs=2,
                    name=f"q1_{rep}_{i}_{half}")
    q1v = q1[:, 0 : no * 16].rearrange("p (o d) -> p o d", d=16)
    nc.vector.tensor_add(q1v, p3[:, :, 0:16], p3[:, :, 16:32])
    q2 = wpool.tile([128, O_PER * 8], f16, tag="q2", bufs=2,
                    name=f"q2_{rep}_{i}_{half}")
    q2v = q2[:, 0 : no * 8].rearrange("p (o d) -> p o d", d=8)
    nc.vector.tensor_add(q2v, q1v[:, :, 0:8], q1v[:, :, 8:16])
    q3 = wpool.tile([128, O_PER * 4], f16, tag="q3", bufs=2,
                    name=f"q3_{rep}_{i}_{half}")
    q3v = q3[:, 0 : no * 4].rearrange("p (o d) -> p o d", d=4)
    nc.vector.tensor_add(q3v, q2v[:, :, 0:4], q2v[:, :, 4:8])
    q4 = wpool.tile([128, O_PER * 2], f16, tag="q4", bufs=2,
                    name=f"q4_{rep}_{i}_{half}")
    q4v = q4[:, 0 : no * 2].rearrange("p (o d) -> p o d", d=2)
    nc.vector.tensor_add(q4v, q3v[:, :, 0:2], q3v[:, :, 2:4])
    ot = opool.tile([128, O_PER], f32, tag="ot", name=f"ot{rep}_{i}_{half}")
    ots = ot[:, 0:no]
    otv = ots.rearrange("p (o d) -> p o d", d=1)
    nc.vector.tensor_add(otv, q4v[:, :, 0:1], q4v[:, :, 1:2])
    nc.vector.tensor_add(ots, ots, brep[:, ob : ob + no])
    nc.gpsimd.dma_start(y[i * BT : (i + 1) * BT, ob : ob + no], ots)


def _emit_body(nc, tc, x, w, b, y, rep=0):
    """Emit one full per-core kernel inside an open TileContext."""
    f32 = mybir.dt.float32
    f16 = mybir.dt.float16
    with (
        tc.tile_pool(name=f"const{rep}", bufs=1) as cpool,
        tc.tile_pool(name=f"work{rep}", bufs=3) as wpool,
        tc.tile_pool(name=f"outp{rep}", bufs=3) as opool,
    ):
        wrep = cpool.tile([128, F_PER], f16, name=f"wrep{rep}")
        brep = cpool.tile([128, O_PER], f32, name=f"brep{rep}")
        nc.gpsimd.dma_start(wrep[:], w[:, :])
        nc.gpsimd.dma_start(brep[:], b[:, :])

        for i in range(N_BT):
            xt32 = wpool.tile([128, F_PER], f32, tag="xt32", bufs=3,
                              name=f"xt32_{rep}_{i}")
            nc.sync.dma_start(xt32[:], x[i * BT : (i + 1) * BT, :])
            if i in SPLIT_TILES:
                _emit_segment(nc, wpool, opool, wrep, brep, y, xt32,
                              rep, i, 0, 0, F_PER // 2)
                _emit_segment(nc, wpool, opool, wrep, brep, y, xt32,
                              rep, i, 1, F_PER // 2, F_PER)
            else:
                _emit_segment(nc, wpool, opool, wrep, brep, y, xt32,
                              rep, i, 0, 0, F_PER)


def _build_program(n_reps=1):
    f32 = mybir.dt.float32
    f16 = mybir.dt.float16
    nc = bass.Bass()
    x = nc.dram_tensor("x", [BATCH, F_PER], f32, kind="ExternalInput")
    w = nc.dram_tensor("w", [128, F_PER], f16, kind="ExternalInput")
    b = nc.dram_tensor("b", [128, O_PER], f32, kind="ExternalInput")
    y = nc.dram_tensor("y", [BATCH, O_PER], f32, kind="ExternalOutput")
    for rep in range(n_reps):
        with TileContext(nc) as tc:
            _emit_body(nc, tc, x, w, b, y, rep=rep)
    return nc


def _finalize(nc):
    data = _split_multi_waits(nc.to_json_bytes())
    nc.to_json_bytes = lambda: data
    return nc


_CACHED = None


def _get_program():
    global _CACHED
    if _CACHED is None:
        _patch_walrus_flags()
        _CACHED = _finalize(_build_program())
    return _CACHED


def _shard_inputs(x, weight, bias):
    x = np.ascontiguousarray(np.asarray(x, dtype=np.float32))
    weight = np.asarray(weight, dtype=np.float32)
    bias = np.asarray(bias, dtype=np.float32)
    assert x.shape == (BATCH, OUT_DIM * DPC) and weight.shape == (OUT_DIM, OUT_DIM * DPC)
    # Diagonal strip of weight: wd[o, d] = weight[o, o*DPC + d]  [OUT_DIM, DPC]
    w3 = weight.reshape(OUT_DIM, OUT_DIM, DPC)
    wd = w3[np.arange(OUT_DIM), np.arange(OUT_DIM)].astype(np.float16)
    in_maps = []
    for k in range(N_CORES):
        fs = slice(k * F_PER, (k + 1) * F_PER)
        os_ = slice(k * O_PER, (k + 1) * O_PER)
        wk = wd[os_].reshape(1, F_PER)
        bk = bias[os_].reshape(1, O_PER)
        in_maps.append(
            {
                "x": np.ascontiguousarray(x[:, fs]),
                "w": np.ascontiguousarray(np.broadcast_to(wk, (128, F_PER))),
                "b": np.ascontiguousarray(np.broadcast_to(bk, (128, O_PER))),
            }
        )
    return in_maps


def kernel(x, weight, bias):
    nc = _get_program()
    in_maps = _shard_inputs(x, weight, bias)
    res = run_bass_kernel_spmd(nc, in_maps, list(range(N_CORES))).results
    return np.concatenate([res[k]["y"] for k in range(N_CORES)], axis=1)


if __name__ == "__main__":
    rng = np.random.default_rng(0)
    x = rng.standard_normal((BATCH, OUT_DIM * DPC), dtype=np.float32)
    w = rng.standard_normal((OUT_DIM, OUT_DIM * DPC), dtype=np.float32)
    b_ = rng.standard_normal(OUT_DIM).astype(np.float32)
    out = kernel(x, w, b_)
    xb = x.reshape(BATCH, OUT_DIM, DPC)
    wb = np.stack([w[o, o * DPC : (o + 1) * DPC] for o in range(OUT_DIM)])
    exp = np.einsum("bod,od->bo", xb, wb) + b_
    rel = np.linalg.norm(out - exp) / np.linalg.norm(exp)
    print("rel err:", rel)


# revision 16
# speedup vs baseline: 1.1272x; 1.1272x over previous
"""Trainium2 Bass kernel for nn_DendriteOutput.

Math: out[b, o] = sum_{d<32} x[b, o*32+d] * weight[o, o*32+d] + bias[o]
(block-diagonal connectivity: only the diagonal 32-wide blocks of `weight`
are touched, so the kernel never reads the other 99.2% of the matrix).

Sharding (8 cores, tensor-parallel over out_dim):
  core k handles outputs [k*256, (k+1)*256) for the full batch, i.e. the
  x column-slab [:, k*8192:(k+1)*8192] (32 MB/core -> the dominant HBM
  traffic; measured stream rate ~400 GB/s/core -> ~87 us for the loads).

Engine/queue layout (per core). Key measured fact: ONE HWDGE ring
streams ~400 GB/s, but the two rings (qSP + qAct) run in parallel ->
~800 GB/s aggregate (35 MB of loads measured at 43 us). So:
  sync HWDGE ring   : x-load segments 0, 2, 4, ... (pure loads, no
                      instruction that waits on compute -- HWDGE DMAs
                      execute FIFO per issuing engine).
  scalar HWDGE ring : x-load segments 1, 3, 5, ... (the ACT sequencer
                      interleaves these doorbells with its casts; the
                      doorbells' buffer-recycle waits trail the cast
                      pipeline by 3 tiles so they never block).
  gpsimd SWDGE      : wrep/brep preloads + the y stores (idle ring; its
                      ot-waits can't stall any load stream).
  ScalarE           : all f32->fp16 tile casts (measured ~2 us per
                      [128, 8192] tile: ACT copies run 4 elem/cyc/lane).
  DVE               : in-place fp16 multiply by the replicated diagonal-
                      weight strip, then a log-tree segmented reduction
                      32->16->8->4->2->1 (fp16 2x strided adds, last
                      level + bias add in f32).  ~6.2 us/tile -> ~50 us
                      total; with the 2-ring stream at ~43 us DVE is the
                      critical path.
First/last batch tiles are split into feature-half segments (own loads)
to shrink pipeline fill and tail.
"""

import json

import numpy as np

import concourse.bass as bass
import concourse.bass_utils as _bass_utils
import concourse.mybir as mybir
from concourse.tile import TileContext
from concourse.bass_utils import run_bass_kernel_spmd

BATCH = 1024
OUT_DIM = 2048
DPC = 32
N_CORES = 8
O_PER = OUT_DIM // N_CORES          # 256 outputs per core
F_PER = O_PER * DPC                 # 8192 features per core
BT = 128                            # batch rows per tile (SBUF partitions)
N_BT = BATCH // BT                  # 8 batch tiles per core
ACT_TILES = set(range(N_BT))        # tiles whose cast runs on ScalarE (all:
                                    # measured ~1.5 us/tile, ACT is idle else)
SPLIT_TILES = {0, N_BT - 1}         # tiles split into feature halves

# ---------------------------------------------------------------------------
# Environment workarounds (in-process only; nothing on disk is modified).
#
# The walrus build in this container (a) needs --dge-levels to lower HWDGE
# DMAs with sem waits (otherwise they hit the V2 pseudo-DMA path that allows
# none) and (b) caps sync waits at ONE per instruction while Tile attaches up
# to N (e.g. the kernel-tail drain). We add the flag and rewrite the
# serialized BIR: extra waits are hoisted into preceding single-wait Drain
# carriers on the same engine (safe: a wait only moves earlier within the
# same engine-program order).
# ---------------------------------------------------------------------------

_patched = False


def _patch_walrus_flags():
    global _patched
    if _patched:
        return
    _patched = True
    orig_rc = _bass_utils.run_command

    def rc(cmd, cwd=None, **kw):
        if cmd and "walrus_driver" in str(cmd[0]):
            cmd = list(cmd)
            cmd.insert(1, "--dge-levels=io,spill_reload,scalar_dynamic_offset")
        return orig_rc(cmd, cwd=cwd, **kw)

    _bass_utils.run_command = rc


def _split_multi_waits(bir_bytes: bytes, cap: int = 1) -> bytes:
    m = json.loads(bir_bytes)
    for fn in m["functions"]:
        for blk in fn["blocks"]:
            out = []
            for inst in blk["instructions"]:
                si = inst.get("sync_info")
                waits = (si or {}).get("on_wait") or []
                if len(waits) > cap:
                    keep = waits[-cap:]
                    for j, wchunk in enumerate(waits[:-cap]):
                        out.append(
                            {
                                "debug": inst.get("debug"),
                                "engine": inst["engine"],
                                "ins": [],
                                "name": f"{inst['name']}-ws{j}",
                                "opcode": "Drain",
                                "outs": [],
                                "sync_info": {
                                    "on_update": [],
                                    "on_wait": [wchunk],
                                },
                            }
                        )
                    si["on_wait"] = keep
                out.append(inst)
            blk["instructions"] = out
    return json.dumps(m).encode()


def _emit_segment(nc, wpool, opool, wrep, brep, y, xt32, rep, i, half, c0, c1):
    """Cast+multiply+reduce+store for columns [c0, c1) of batch tile i.

    xt32: the f32 x segment tile holding columns [c0, c1) of row block i
    (shape [128, c1-c0]). half: unique name suffix.
    """
    f32 = mybir.dt.float32
    f16 = mybir.dt.float16
    nf = c1 - c0
    no = nf // DPC
    ob = c0 // DPC
    q0 = wpool.tile([128, F_PER], f16, tag="q0", bufs=2,
                    name=f"q0_{rep}_{i}_{half}")
    q0s = q0[:, 0:nf]
    nc.scalar.copy(q0s, xt32[:])
    nc.vector.tensor_mul(q0s, q0s, wrep[:, c0:c1])
    p3 = q0s.rearrange("p (o d) -> p o d", d=DPC)
    q1 = wpool.tile([128, O_PER * 16], f16, tag="q1", bufs=1,
                    name=f"q1_{rep}_{i}_{half}")
    q1v = q1[:, 0 : no * 16].rearrange("p (o d) -> p o d", d=16)
    nc.vector.tensor_add(q1v, p3[:, :, 0:16], p3[:, :, 16:32])
    q2 = wpool.tile([128, O_PER * 8], f16, tag="q2", bufs=1,
                    name=f"q2_{rep}_{i}_{half}")
    q2v = q2[:, 0 : no * 8].rearrange("p (o d) -> p o d", d=8)
    nc.vector.tensor_add(q2v, q1v[:, :, 0:8], q1v[:, :, 8:16])
    q3 = wpool.tile([128, O_PER * 4], f16, tag="q3", bufs=1,
                    name=f"q3_{rep}_{i}_{half}")
    q3v = q3[:, 0 : no * 4].rearrange("p (o d) -> p o d", d=4)
    nc.vector.tensor_add(q3v, q2v[:, :, 0:4], q2v[:, :, 4:8])
    q4 = wpool.tile([128, O_PER * 2], f16, tag="q4", bufs=1,
                    name=f"q4_{rep}_{i}_{half}")
    q4v = q4[:, 0 : no * 2].rearrange("p (o d) -> p o d", d=2)
    nc.vector.tensor_add(q4v, q3v[:, :, 0:2], q3v[:, :, 2:4])
    ot = opool.tile([128, O_PER], f32, tag="ot", name=f"ot{rep}_{i}_{half}")
    ots = ot[:, 0:no]
    otv = ots.rearrange("p (o d) -> p o d", d=1)
    nc.vector.tensor_add(otv, q4v[:, :, 0:1], q4v[:, :, 1:2])
    nc.vector.tensor_add(ots, ots, brep[:, ob : ob + no])
    nc.gpsimd.dma_start(y[i * BT : (i + 1) * BT, ob : ob + no], ots)


def _emit_body(nc, tc, x, w, b, y, rep=0):
    """Emit one full per-core kernel inside an open TileContext."""
    f32 = mybir.dt.float32
    f16 = mybir.dt.float16
    with (
        tc.tile_pool(name=f"const{rep}", bufs=1) as cpool,
        tc.tile_pool(name=f"work{rep}", bufs=3) as wpool,
        tc.tile_pool(name=f"outp{rep}", bufs=3) as opool,
    ):
        wrep = cpool.tile([128, F_PER], f16, name=f"wrep{rep}")
        brep = cpool.tile([128, O_PER], f32, name=f"brep{rep}")
        nc.gpsimd.dma_start(wrep[:], w[:, :])
        nc.gpsimd.dma_start(brep[:], b[:, :])

        # Segments: (row_block, c0, c1). First/last tiles split into
        # feature halves; loads alternate between the two HWDGE rings.
        segs = []
        for i in range(N_BT):
            if i in SPLIT_TILES:
                segs.append((i, 0, F_PER // 2))
                segs.append((i, F_PER // 2, F_PER))
            else:
                segs.append((i, 0, F_PER))
        for s, (i, c0, c1) in enumerate(segs):
            nf = c1 - c0
            tag = "xt32h" if nf < F_PER else "xt32"
            nbufs = 2 if nf < F_PER else 3
            xt32 = wpool.tile([128, nf], f32, tag=tag, bufs=nbufs,
                              name=f"xt32_{rep}_{i}_{c0}")
            eng = nc.sync if s % 2 == 0 else nc.scalar
            eng.dma_start(xt32[:], x[i * BT : (i + 1) * BT, c0:c1])
            _emit_segment(nc, wpool, opool, wrep, brep, y, xt32,
                          rep, i, c0 // F_PER if nf == F_PER else c0,
                          c0, c1)


def _build_program(n_reps=1):
    f32 = mybir.dt.float32
    f16 = mybir.dt.float16
    nc = bass.Bass()
    x = nc.dram_tensor("x", [BATCH, F_PER], f32, kind="ExternalInput")
    w = nc.dram_tensor("w", [128, F_PER], f16, kind="ExternalInput")
    b = nc.dram_tensor("b", [128, O_PER], f32, kind="ExternalInput")
    y = nc.dram_tensor("y", [BATCH, O_PER], f32, kind="ExternalOutput")
    for rep in range(n_reps):
        with TileContext(nc) as tc:
            _emit_body(nc, tc, x, w, b, y, rep=rep)
    return nc


def _finalize(nc):
    data = _split_multi_waits(nc.to_json_bytes())
    nc.to_json_bytes = lambda: data
    return nc


_CACHED = None


def _get_program():
    global _CACHED
    if _CACHED is None:
        _patch_walrus_flags()
        _CACHED = _finalize(_build_program())
    return _CACHED


def _shard_inputs(x, weight, bias):
    x = np.ascontiguousarray(np.asarray(x, dtype=np.float32))
    weight = np.asarray(weight, dtype=np.float32)
    bias = np.asarray(bias, dtype=np.float32)
    assert x.shape == (BATCH, OUT_DIM * DPC) and weight.shape == (OUT_DIM, OUT_DIM * DPC)
    # Diagonal strip of weight: wd[o, d] = weight[o, o*DPC + d]  [OUT_DIM, DPC]
    w3 = weight.reshape(OUT_DIM, OUT_DIM, DPC)
    wd = w3[np.arange(OUT_DIM), np.arange(OUT_DIM)].astype(np.float16)
    in_maps = []
    for k in range(N_CORES):
        fs = slice(k * F_PER, (k + 1) * F_PER)
        os_ = slice(k * O_PER, (k + 1) * O_PER)
        wk = wd[os_].reshape(1, F_PER)
        bk = bias[os_].reshape(1, O_PER)
        in_maps.append(
            {
                "x": np.ascontiguousarray(x[:, fs]),
                "w": np.ascontiguousarray(np.broadcast_to(wk, (128, F_PER))),
                "b": np.ascontiguousarray(np.broadcast_to(bk, (128, O_PER))),
            }
        )
    return in_maps


def kernel(x, weight, bias):
    nc = _get_program()
    in_maps = _shard_inputs(x, weight, bias)
    res = run_bass_kernel_spmd(nc, in_maps, list(range(N_CORES))).results
    return np.concatenate([res[k]["y"] for k in range(N_CORES)], axis=1)


if __name__ == "__main__":
    rng = np.random.default_rng(0)
    x = rng.standard_normal((BATCH, OUT_DIM * DPC), dtype=np.float32)
    w = rng.standard_normal((OUT_DIM, OUT_DIM * DPC), dtype=np.float32)
    b_ = rng.standard_normal(OUT_DIM).astype(np.float32)
    out = kernel(x, w, b_)
    xb = x.reshape(BATCH, OUT_DIM, DPC)
    wb = np.stack([w[o, o * DPC : (o + 1) * DPC] for o in range(OUT_DIM)])
    exp = np.einsum("bod,od->bo", xb, wb) + b_
    rel = np.linalg.norm(out - exp) / np.linalg.norm(exp)
    print("rel err:", rel)
